# revision 5
# baseline (speedup 1.0000x reference)
"""Trainium2 Bass kernel for nn_BoundingBoxDiscipline (loss_fn).

Strategy: pure data parallel over the batch — 32 samples -> 8 cores x 4.
Per core, each (tensor, sample, 128-row block) chunk [128, 512, 21] f32 is
DMA'd to SBUF (5.25 MiB contiguous, partition = image row). The DVE then:
  1. rmax = reduce_max over the 21 channels (grouped 3D reduce, axis=X)
  2. m    = (rmax > p[..,0])  fused with  any_row = max(m)      (TTR)
  3.        (x-512)*m         fused with  row_xmin' = min(...)  (TTR)
  4.        (x+1)*m           fused with  row_xmax' = max(...)  (TTR)
mask == (argmax over channels > 0) exactly (incl. first-max tie semantics),
and all coordinate arithmetic is exact in f32 (values < 2^10).

The per-core result is a tiny [2, 4, 128, 12] tensor of per-row stats; the
host reconstructs the per-sample bounding boxes and evaluates the scalar
penalty in float32 numpy, mirroring the reference op-for-op.
"""

import numpy as np

_TRN_REPO = "/opt/trn_rl_repo"

B, H, W, C = 32, 512, 512, 21
N_CORES = 8
BL = B // N_CORES  # samples per core
PR = 128           # SBUF partitions == image rows per block
RB = H // PR       # row blocks per sample
PENALTY_WEIGHT = np.float32(0.05)

_cache = {}
_last_results = None  # BassKernelResults of the most recent run (for profiling)


def _ensure_path():
    import sys

    if _TRN_REPO not in sys.path:
        sys.path.insert(0, _TRN_REPO)


def _install_walrus_wait_fixup():
    """This container's walrus_driver rejects instructions carrying more than
    one semaphore wait ("Too many sync wait commands", CoreV3GenImpl:104).
    Split the extra waits onto single-wait Drain instructions inserted just
    before the offending instruction on the same engine — same-engine
    program order makes the chain semantically identical to the multi-wait."""
    import orjson

    import concourse.bass as bass

    if getattr(bass.Bass.to_json_bytes, "_wait_split", False):
        return
    orig = bass.Bass.to_json_bytes

    def to_json_bytes(self):
        data = orjson.loads(orig(self))
        n = 0
        for fn in data.get("functions", []):
            for blk in fn.get("blocks", []):
                out = []
                for inst in blk.get("instructions", []):
                    si = inst.get("sync_info") or {}
                    ow = si.get("on_wait") or []
                    if len(ow) > 1:
                        for w_ in ow[:-1]:
                            n += 1
                            out.append(
                                {
                                    "debug": inst.get("debug", 0),
                                    "engine": inst["engine"],
                                    "ins": [],
                                    "name": f"waitsplit-{n}",
                                    "opcode": "Drain",
                                    "outs": [],
                                    "sync_info": {"on_update": [], "on_wait": [w_]},
                                }
                            )
                        si = dict(si)
                        si["on_wait"] = [ow[-1]]
                        inst = dict(inst)
                        inst["sync_info"] = si
                    out.append(inst)
                blk["instructions"] = out
        return orjson.dumps(data)

    to_json_bytes._wait_split = True
    bass.Bass.to_json_bytes = to_json_bytes


def _build_nc(bl=BL, rb=RB, w=W, c=C, data_bufs=3):
    _ensure_path()
    import concourse.bass as bass
    import concourse.tile as tile
    from concourse import mybir

    _install_walrus_wait_fixup()

    f32 = mybir.dt.float32
    nc = bass.Bass()
    pred_d = nc.dram_tensor("pred", [bl, rb, PR, w, c], f32, kind="ExternalInput")
    exp_d = nc.dram_tensor("exp", [bl, rb, PR, w, c], f32, kind="ExternalInput")
    iota_d = nc.dram_tensor("iota", [PR, 2 * w], f32, kind="ExternalInput")
    res_d = nc.dram_tensor("res", [2, bl, PR, 2 * rb], f32, kind="ExternalOutput")

    with tile.TileContext(nc) as tc:
        with tc.tile_pool(name="consts", bufs=1) as consts, \
             tc.tile_pool(name="data", bufs=data_bufs) as data, \
             tc.tile_pool(name="small", bufs=3) as small, \
             tc.tile_pool(name="resp", bufs=2) as resp:
            iota_sb = consts.tile([PR, 2 * w], f32)
            nc.sync.dma_start(out=iota_sb[:, :], in_=iota_d[:, :])
            for t, td in enumerate((pred_d, exp_d)):
                for s in range(bl):
                    res_tile = resp.tile([PR, 2 * rb], f32)
                    for r in range(rb):
                        dtile = data.tile([PR, w, c], f32)
                        nc.sync.dma_start(out=dtile[:, :, :], in_=td[s, r])
                        rmax = small.tile([PR, w], f32)
                        nc.vector.reduce_max(
                            rmax[:, :], dtile[:, :, :], axis=mybir.AxisListType.X
                        )
                        m = small.tile([PR, w], f32)
                        nc.vector.tensor_tensor(
                            m[:, :], rmax[:, :], dtile[:, :, 0],
                            op=mybir.AluOpType.is_gt,
                        )
                        vmin = small.tile([PR, w], f32)
                        nc.vector.tensor_tensor(
                            vmin[:, :], m[:, :], iota_sb[:, :w],
                            op=mybir.AluOpType.mult,
                        )
                        nc.vector.tensor_reduce(
                            res_tile[:, 2 * r : 2 * r + 1], vmin[:, :],
                            axis=mybir.AxisListType.X, op=mybir.AluOpType.min,
                        )
                        vmax = small.tile([PR, w], f32)
                        nc.vector.tensor_tensor(
                            vmax[:, :], m[:, :], iota_sb[:, w:],
                            op=mybir.AluOpType.mult,
                        )
                        nc.vector.tensor_reduce(
                            res_tile[:, 2 * r + 1 : 2 * r + 2], vmax[:, :],
                            axis=mybir.AxisListType.X, op=mybir.AluOpType.max,
                        )
                    nc.sync.dma_start(out=res_d[t, s], in_=res_tile[:, :])
    return nc


def _iota_const(w=W):
    x = np.arange(w, dtype=np.float32)
    out = np.empty((PR, 2 * w), np.float32)
    out[:, :w] = x - np.float32(w)     # x - 512 : masked-min operand
    out[:, w:] = x + np.float32(1.0)   # x + 1   : masked-max operand
    return out


def _boxes_from_stats(res):
    """res: [N_CORES, 2, BL, PR, 2*RB] -> boxes [2, B, 4] f32, has [2, B] bool.

    Per row: col 2r   = min((x-512)*m)  -> xmin-512, or 0 if row empty
             col 2r+1 = max((x+1)*m)    -> xmax+1,   or 0 if row empty
    """
    A = (
        res.reshape(N_CORES, 2, BL, PR, RB, 2)
        .transpose(1, 0, 2, 4, 3, 5)  # -> [t, core, s, r, p, k]
        .reshape(2, B, H, 2)          # row index = 128*r + p
    )
    anyr = A[..., 1] > 0.5  # [2, B, H] : row has mask iff xmax+1 >= 1
    has = anyr.any(axis=2)  # [2, B]
    ymin = np.argmax(anyr, axis=2).astype(np.float32)
    ymax = np.float32(H - 1) - np.argmax(anyr[:, :, ::-1], axis=2).astype(np.float32)
    xmin = A[..., 0].min(axis=2).astype(np.float32) + np.float32(W)
    xmax = A[..., 1].max(axis=2).astype(np.float32) - np.float32(1.0)
    boxes = np.stack([ymin, xmin, ymax, xmax], axis=-1).astype(np.float32)
    fallback = np.array([0.0, 0.0, 1.0, 1.0], dtype=np.float32)
    boxes = np.where(has[..., None], boxes, fallback).astype(np.float32)
    return boxes, has


def _penalty(boxes, has):
    p_box, t_box = boxes[0], boxes[1]
    has_p, has_t = has[0], has[1]
    pred_area = (p_box[:, 2] - p_box[:, 0] + 1.0) * (p_box[:, 3] - p_box[:, 1] + 1.0)
    true_area = (t_box[:, 2] - t_box[:, 0] + 1.0) * (t_box[:, 3] - t_box[:, 1] + 1.0)
    area_penalty = np.maximum(pred_area - true_area, 0.0) / (true_area + 1.0)
    center_offset = np.sqrt(
        np.square((p_box[:, 0] + p_box[:, 2]) / 2.0 - (t_box[:, 0] + t_box[:, 2]) / 2.0)
        + np.square((p_box[:, 1] + p_box[:, 3]) / 2.0 - (t_box[:, 1] + t_box[:, 3]) / 2.0)
    ) / np.float32(20.0)
    inter_ymin = np.maximum(p_box[:, 0], t_box[:, 0])
    inter_xmin = np.maximum(p_box[:, 1], t_box[:, 1])
    inter_ymax = np.minimum(p_box[:, 2], t_box[:, 2])
    inter_xmax = np.minimum(p_box[:, 3], t_box[:, 3])
    inter_area = np.maximum(np.float32(0.0), inter_ymax - inter_ymin + 1.0) * np.maximum(
        np.float32(0.0), inter_xmax - inter_xmin + 1.0
    )
    union_area = pred_area + true_area - inter_area + np.float32(1e-6)
    iou_penalty = np.float32(1.0) - inter_area / union_area
    total_penalty = (area_penalty + center_offset + iou_penalty).astype(np.float32)
    penalties = np.where(has_t & has_p, np.tanh(total_penalty), np.float32(0.0)).astype(
        np.float32
    )
    return np.array(PENALTY_WEIGHT * penalties.mean(dtype=np.float32), dtype=np.float32)


def kernel(prediction_probs, expected_onehot):
    _ensure_path()
    from concourse.bass_utils import run_bass_kernel_spmd

    global _last_results
    if "nc" not in _cache:
        _cache["nc"] = _build_nc()
    nc = _cache["nc"]

    pred = np.ascontiguousarray(prediction_probs, dtype=np.float32).reshape(
        N_CORES, BL, RB, PR, W, C
    )
    exp_ = np.ascontiguousarray(expected_onehot, dtype=np.float32).reshape(
        N_CORES, BL, RB, PR, W, C
    )
    iota = _iota_const()
    in_maps = [
        {"pred": pred[cc], "exp": exp_[cc], "iota": iota} for cc in range(N_CORES)
    ]
    r = run_bass_kernel_spmd(nc, in_maps, list(range(N_CORES)))
    _last_results = r
    res = np.stack([r.results[cc]["res"] for cc in range(N_CORES)])
    _cache["last_res_stats"] = res
    boxes, has = _boxes_from_stats(res)
    return _penalty(boxes, has)
